# revision 14
# baseline (speedup 1.0000x reference)
"""Interleaved 2x2 upsample kernel for Trainium2 (8 NeuronCores, SPMD).

Input  x: (16, 3, 1024, 1024) f32
Output y: (16, 1, 2048, 2048) f32 where
  y[b, 0, 2i,   2j  ] = x[b, 0, i, j]
  y[b, 0, 2i,   2j+1] = x[b, 1, i, j]
  y[b, 0, 2i+1, 2j  ] = x[b, 2, i, j]
  y[b, 0, 2i+1, 2j+1] = -1

Sharding: pure data parallel over batch (2 batches per core).

Per-core kernel: pure data movement, HBM-bandwidth-bound (56 MiB/core).
The 16 per-core DMA engines stream at a flat ~26.9 GB/s each (16 B/cycle),
so the floor is 58.7 MB / 430 GB/s ~= 137us of engine work + ~8us NEFF
preamble + ~3us epilogue. Each iteration covers u*128 input rows,
partition p holding u consecutive rows per channel (channel-outer layout
-> u*4 KiB contiguous DRAM runs on the load). Three strided on-chip
copies (DVE / GpSimd / ACT, one each so the copy latency is a single
copy) build the 2x2 interleave in an output tile where partition p holds
2u consecutive output rows (u*16 KiB contiguous store runs); constant -1
columns are memset once per buffer. Loads and stores are issued on ONE
hardware DMA queue (sync/SP) in software-pipelined order (loads NSRC
iterations ahead), so all 16 DMA engines process the identical FIFO and
stay in lock-step; the end of the schedule tapers to u=1. Measured:
~150.5-152us per core, engines gap-free (the residual ~181us outlier mode
is a single externally-degraded DMA engine at ~22 GB/s, visible in traces
as one straggler engine with inflated packet durations).
"""

import numpy as np

B, C, H, W = 16, 3, 1024, 1024
N_CORES = 8
B_PER_CORE = B // N_CORES  # 2
P = 128                    # SBUF partitions
UMAX = 2                   # max 128-row units per iteration
NSRC = 3                   # src ring depth
NOUT = 4                   # out ring depth: extra slack so copies never wait
                           # on a store-completion semaphore (those sometimes
                           # take ~20us to propagate and convoy the pipeline)

# per-batch iteration sizes, in 128-row units (must sum to H // P = 8).
# Engines are load-saturated from the first issue, so no start taper; the
# end tapers to u=1 to keep the final load block small -- with an all-u2
# schedule the last iterations' store issues intermittently convoy (~20us
# semaphore stalls, +30us total), so the taper is load-bearing.
SCHED = {0: [2, 2, 2, 2], 1: [2, 2, 2, 1, 1]}

_CACHE = {}


def _build():
    import concourse.bacc as bacc
    import concourse.mybir as mybir
    import concourse.tile as tile

    f32 = mybir.dt.float32
    nc = bacc.Bacc("TRN2", target_bir_lowering=False, debug=False)

    x = nc.dram_tensor("x", [B_PER_CORE, C, H, W], f32, kind="ExternalInput")
    y = nc.dram_tensor("y", [B_PER_CORE, 1, 2 * H, 2 * W], f32, kind="ExternalOutput")

    with tile.TileContext(nc) as tc:
        with tc.tile_pool(name="io", bufs=1) as pool:
            srcs = [
                pool.tile([P, UMAX * C * W], f32, name=f"src{k}", tag=f"src{k}")
                for k in range(NSRC)
            ]
            outs = [
                pool.tile([P, UMAX * 4 * W], f32, name=f"out{k}", tag=f"out{k}")
                for k in range(NOUT)
            ]

            # Constant -1 columns (odd output row, odd output col): written
            # once per buffer, never clobbered. Covers the u=1 prefix too.
            for k in range(NOUT):
                ov = outs[k][:].rearrange(
                    "p (r e j q) -> p r e j q", r=UMAX, e=2, j=W
                )
                nc.gpsimd.memset(ov[:, :, 1, :, 1], -1.0)

            # Flatten the schedule into (batch, row0, u) iterations.
            iters = []
            for b in range(B_PER_CORE):
                row0 = 0
                for u in SCHED[b]:
                    iters.append((b, row0, u))
                    row0 += P * u

            def emit_load(t):
                b, row0, u = iters[t]
                src = srcs[t % NSRC]
                # Load: partition p <- rows [row0+u*p, row0+u*(p+1)) of
                # each channel; channel-outer so each (p, c) run is
                # u*4096 B contiguous in DRAM.
                sv = src[:, : u * C * W].rearrange(
                    "p (c r j) -> p c r j", c=C, r=u
                )
                xin = x[b][:, row0 : row0 + P * u, :].rearrange(
                    "c (p r) w -> p c r w", r=u
                )
                nc.sync.dma_start(out=sv, in_=xin)

            def emit_interleave_store(t):
                b, row0, u = iters[t]
                src = srcs[t % NSRC]
                out = outs[t % NOUT]
                sv = src[:, : u * C * W].rearrange(
                    "p (c r j) -> p c r j", c=C, r=u
                )
                # Interleave into the output tile: partition p holds
                # output rows [2*(row0+u*p), 2*(row0+u*p) + 2u); one copy
                # per engine so the copy latency is one copy, not three.
                ov = out[:, : u * 4 * W].rearrange(
                    "p (r e j q) -> p r e j q", r=u, e=2, j=W
                )
                nc.vector.tensor_copy(ov[:, :, 0, :, 0], sv[:, 0])
                nc.gpsimd.tensor_copy(ov[:, :, 1, :, 0], sv[:, 2])
                nc.scalar.copy(ov[:, :, 0, :, 1], sv[:, 1])

                # Store: u*16 KiB contiguous per partition on both sides.
                yout = y[b, 0][2 * row0 : 2 * (row0 + P * u), :].rearrange(
                    "(p f) w -> p (f w)", f=2 * u
                )
                nc.sync.dma_start(out=yout, in_=out[:, : u * 4 * W])

            # Software-pipelined issue order, loads LA iterations ahead, and
            # loads AND stores on the same hw queue (sync/SP). Every DMA
            # engine then sees one FIFO with the identical deterministic
            # load/store interleave, so engines cannot diverge in local
            # queue arbitration. (With separate load/store queues, engines
            # intermittently straggle ~40us apart, and every all-16-engine
            # completion semaphore then waits on the straggler - observed
            # as ~20us pipeline convoys costing +30us end-to-end.) The
            # lookahead keeps sync's blocking wait for iteration k's copies
            # from delaying the issue of load k+1..k+LA.
            # Emission order within a step matters: iteration t-LA's copies
            # must be emitted BEFORE load t (same src buffer, t % NSRC ==
            # (t-LA) % NSRC) so the copies chain to load t-LA's data and
            # load t chains WAR-correctly behind the copies.
            LA = NSRC
            for t in range(len(iters) + LA):
                if t >= LA:
                    emit_interleave_store(t - LA)
                if t < len(iters):
                    emit_load(t)

    nc.finalize()
    return nc


def _get_nc():
    if "nc" not in _CACHE:
        _CACHE["nc"] = _build()
    return _CACHE["nc"]


def kernel(x):
    from concourse.bass_utils import run_bass_kernel_spmd

    x = np.ascontiguousarray(np.asarray(x), dtype=np.float32)
    assert x.shape == (B, C, H, W), x.shape

    nc = _get_nc()
    in_maps = [
        {"x": np.ascontiguousarray(x[i * B_PER_CORE : (i + 1) * B_PER_CORE])}
        for i in range(N_CORES)
    ]
    res = run_bass_kernel_spmd(nc, in_maps, list(range(N_CORES))).results
    return np.concatenate([res[i]["y"] for i in range(N_CORES)], axis=0)

